# revision 7
# baseline (speedup 1.0000x reference)
"""Trainium2 Bass kernel for nn_CausalAttention (gated-resnet q/k/v projections
+ causal attention). Data-parallel over batch: 8 batches -> 8 NeuronCores.

Per-core computation (batch b), all fp32 storage:
  x_q = query[b] (C=256, S=1024)   x_k = key[b] (256, 1024)
  branch(p, x): e  = elu(x)
                h1 = W1 @ e + b1 ; e1 = elu(h1)
                h2 = W2 @ e1 + b2 ; a, g = split(h2)
                gr = x + a * sigmoid(g)
                o  = Wn @ gr + bn          (512, 1024) channel-major
  q = branch(q, x_q); k = branch(k, x_k); v = branch(v, x_k)
  att view: X_att[s, d] = X_cm[s//2, (s%2)*512 + d]  (flat reinterpretation)
  per head n (d = 64n..64n+63):
    scoresT[s2, s1] = sum_d K_att[s2,d] Q_att[s1,d]   (s2 causal blocks)
    eT = exp(scoresT/sqrt(512)) with strict-lower mask (s2 < s1)
    outT[vs, s1] = sum_s2 V_att[s2, 64n+vs] * eT[s2, s1] ; l[s1] = sum_s2 eT
    final[64n+vs, s1] = outT[vs, s1] / l[s1]   (row 0 of l patched to 1)

v2: engine-rebalanced + software-pipelined:
  - branches issued stage-interleaved (h1 v,k,q; e1 v,k,q; ...) so PE
    matmuls of one branch overlap DVE/ACT work of another
  - elu combine / glu mult / gr add / masks moved to gpsimd (was idle)
  - v_aug built by direct SBUF->SBUF DMA (no DRAM roundtrip)
  - softmax normalize: reciprocal (DVE) -> gpsimd partition_broadcast ->
    DVE multiply (no DRAM roundtrips, no big psum->sbuf copy)
"""

import os
import sys
import numpy as np

sys.path.insert(0, "/opt/trn_rl_repo")

C = 256
S = 1024
D = 512
NH = 8
KS = 64
VS = 64
SCALE = 1.0 / float(np.sqrt(512.0))
N_CORES = 8

CFG = {
    "mm_dtype": "bfloat16",  # "float32" | "bfloat16"
    # gpsimd only supports plain tensor_tensor (no scalar-imm ops)
    "elu_combine_engine": "gpsimd",  # dst = me' + r   (me' = min(e,1)-1)
    "glu_mult_engine": "vector",     # u = ha*(1+tg)   (stt needs V)
    "gr_add_engine": "gpsimd",       # gr = u + x
    "mask_engine": "gpsimd",         # eT diag *= mask01
    "fin_engine": "vector",          # fin = pv * rb
    "bcast": "gpsimd",               # rb broadcast: "gpsimd" | "dma"
}


def _split_psum_ranges(a, b, max_n=512):
    """Split [a, b) psum column range into chunks that don't cross 512-col
    bank boundaries and are <= max_n wide."""
    out = []
    while a < b:
        nxt = min(b, ((a // 512) + 1) * 512, a + max_n)
        out.append((a, nxt))
        a = nxt
    return out


def build_program(cfg=CFG):
    from contextlib import ExitStack

    import concourse.bacc as bacc
    import concourse.bass as bass
    import concourse.tile as tile
    from concourse import mybir
    from concourse.alu_op_type import AluOpType as Op

    f32 = mybir.dt.float32
    mmdt = getattr(mybir.dt, cfg["mm_dtype"])
    mdt = mmdt
    AF = mybir.ActivationFunctionType

    nc = bacc.Bacc("TRN2", target_bir_lowering=False, debug=False,
                   num_devices=N_CORES)

    # ---------------- DRAM parameters ----------------
    idt = mybir.dt.bfloat16 if cfg["mm_dtype"] == "bfloat16" else f32
    query = nc.dram_tensor("query", [C, S], idt, kind="ExternalInput").ap()
    key = nc.dram_tensor("key", [C, S], idt, kind="ExternalInput").ap()
    wT = {}
    bias = {}
    wdt = mdt if mdt == mybir.dt.bfloat16 else f32
    for p in ("q", "k", "v"):
        wT[p, 1] = nc.dram_tensor(f"{p}_w1T", [C, C], wdt, kind="ExternalInput").ap()
        wT[p, 2] = nc.dram_tensor(f"{p}_w2T", [C, 2 * C], wdt, kind="ExternalInput").ap()
        wT[p, "n"] = nc.dram_tensor(f"{p}_wnT", [C, D], wdt, kind="ExternalInput").ap()
        bias[p, 1] = nc.dram_tensor(f"{p}_b1", [C], f32, kind="ExternalInput").ap()
        bias[p, 2] = nc.dram_tensor(f"{p}_b2", [2 * C], f32, kind="ExternalInput").ap()
        bias[p, "n"] = nc.dram_tensor(f"{p}_bn", [D], f32, kind="ExternalInput").ap()
    out_d = nc.dram_tensor("out", [D, S], f32, kind="ExternalOutput").ap()

    def eng(name):
        return getattr(nc, name)

    BR = ("v", "k", "q")  # issue order within stages

    with tile.TileContext(nc) as tc, ExitStack() as ctx:
        # ------------- pools -------------
        persist = ctx.enter_context(tc.tile_pool(name="persist", bufs=1))
        psum_main = ctx.enter_context(tc.tile_pool(name="psum_main", bufs=3, space="PSUM"))
        psum_pv = ctx.enter_context(tc.tile_pool(name="psum_pv", bufs=2, space="PSUM"))
        wk = ctx.enter_context(tc.tile_pool(name="wk", bufs=5))
        big = ctx.enter_context(tc.tile_pool(name="big", bufs=1))
        eT_pool = ctx.enter_context(tc.tile_pool(name="eT", bufs=3))
        att_small = ctx.enter_context(tc.tile_pool(name="att_small", bufs=4))

        # persistent tiles
        xq = persist.tile([128, 2, S], idt)
        xk = persist.tile([128, 2, S], idt)
        eluq = persist.tile([128, 2, S], mdt)
        eluk = persist.tile([128, 2, S], mdt)
        qT_m = persist.tile([128, 4, S], mdt)   # Q^T_att: [dd%128, dd//128, s]
        kT_m = persist.tile([128, 4, S], mdt)
        v_aug = persist.tile([128, 8, NH, VS + 1], mdt)  # [s%128, s//128, n, vs|1]
        mask01 = persist.tile([128, 128], mdt)  # [t2, t1] = 1.0 if t1 > t2 else 0

        # PE warm-up: back-to-back matmuls on scratch data during DMA phase
        warm = persist.tile([128, 512], mdt, name="warm")
        nc.vector.memset(warm, 0.5)
        wps = psum_main.tile([128, 1024], f32, tag="pm", name="wps")
        for _ in range(18):
            nc.tensor.matmul(wps[:, 0:512], lhsT=warm[:, 0:128],
                             rhs=warm, start=True, stop=True)

        # inputs
        for cc in range(2):
            nc.sync.dma_start(out=xk[:, cc, :], in_=key[cc * 128:(cc + 1) * 128, :])
            nc.sync.dma_start(out=xq[:, cc, :], in_=query[cc * 128:(cc + 1) * 128, :])
        bnb = {"q": persist.tile([128, D], f32, name="bnb_q"),
               "k": persist.tile([128, D], f32, name="bnb_k")}
        for p in ("q", "k"):
            bn_ap = bias[p, "n"]
            bn_bcast = bass.AP(tensor=bn_ap.tensor, offset=bn_ap.offset,
                               ap=[[0, 128]] + list(bn_ap.ap))
            nc.sync.dma_start(out=bnb[p], in_=bn_bcast)
        # strict-lower mask: keep 1.0 where t1 - t2 - 1 >= 0
        nc.gpsimd.memset(mask01, 1.0)
        nc.gpsimd.affine_select(
            out=mask01, in_=mask01, compare_op=Op.is_ge, fill=0.0,
            base=-1, pattern=[[1, 128]], channel_multiplier=-1,
        )
        # v_aug ones column (independent of everything; do during DMA phase)
        for j in range(8):
            nc.gpsimd.memset(v_aug[:, j, :, VS:VS + 1], 1.0)

        # ------- weights for all branches upfront (persistent) -------
        w1 = {}
        w2 = {}
        wn = {}
        b1 = {}
        b2 = {}
        b2h = {}
        for p in BR:
            w1[p] = persist.tile([128, 2, C], mdt, name=f"w1_{p}")
            w2[p] = persist.tile([128, 2, 2 * C], mdt, name=f"w2_{p}")
            wn[p] = persist.tile([128, 2, D], mdt, name=f"wn_{p}")

            def wcast(ap):
                return ap if ap.dtype == mdt else ap.bitcast(mdt)
            for kc in range(2):
                nc.sync.dma_start(out=w1[p][:, kc, :], in_=wcast(wT[p, 1][kc * 128:(kc + 1) * 128, :]))
                nc.sync.dma_start(out=w2[p][:, kc, :], in_=wcast(wT[p, 2][kc * 128:(kc + 1) * 128, :]))
                nc.sync.dma_start(out=wn[p][:, kc, :], in_=wcast(wT[p, "n"][kc * 128:(kc + 1) * 128, :]))
            b1[p] = persist.tile([128, 2], f32, name=f"b1_{p}")
            b2[p] = persist.tile([128, 4], f32, name=f"b2_{p}")
            b2h[p] = persist.tile([128, 4], f32, name=f"b2h_{p}")
            nc.sync.dma_start(out=b1[p], in_=bias[p, 1].rearrange("(kc p) -> p kc", p=128))
            nc.sync.dma_start(out=b2[p], in_=bias[p, 2].rearrange("(kc p) -> p kc", p=128))
            nc.vector.tensor_scalar_mul(b2h[p], b2[p], 0.5)
        bnv = persist.tile([128, 4], f32, name="bnv")
        nc.sync.dma_start(out=bnv, in_=bias["v", "n"].rearrange("(kc p) -> p kc", p=128))

        # ---------------- branch compute, stage-interleaved ----------------
        # elu(x) for inputs: xk first (feeds v and k), then xq
        def elu_chunk(dst, src_ap, bias_col=None):
            """dst = elu(src) for a (128, S) chunk. src may be psum.
            vector: r = max(src [+b], 0); scalar: e = exp(src [+b]);
            gpsimd: me = min(e, 1); dst = (me - 1) + r."""
            r = wk.tile([128, S], mdt, tag="wkr")
            e = wk.tile([128, S], mdt, tag="wke")
            me = wk.tile([128, S], mdt, tag="wkm")
            if bias_col is None:
                nc.vector.tensor_scalar_max(r, src_ap, 0.0)
                nc.scalar.activation(e, src_ap, AF.Exp)
            else:
                nc.vector.tensor_scalar(r, src_ap, bias_col, 0.0, Op.add, Op.max)
                nc.scalar.activation(e, src_ap, AF.Exp, bias=bias_col)
            # me = min(e, 1) - 1   (one vector op); dst = me + r (gpsimd)
            nc.vector.tensor_scalar(me, e, 1.0, -1.0, Op.min, Op.add)
            eng(cfg["elu_combine_engine"]).tensor_tensor(dst, me, r, Op.add)

        for cc in range(2):
            elu_chunk(eluk[:, cc, :], xk[:, cc, :])
        for cc in range(2):
            elu_chunk(eluq[:, cc, :], xq[:, cc, :])

        elu3 = {"v": eluk, "k": eluk, "q": eluq}
        x3 = {"v": xk, "k": xk, "q": xq}

        # --- stage 1: h1 = W1 @ elu(x) + b1 ; e1 = elu(h1) ---
        e1 = {p: big.tile([128, 2, S], mdt, name=f"e1_{p}") for p in BR}
        for p in BR:
            for mc in range(2):
                ps = psum_main.tile([128, 1024], f32, tag="pm")
                h1 = ps[:, 0:S]
                for nk in range(2):
                    for kc in range(2):
                        nc.tensor.matmul(
                            h1[:, nk * 512:(nk + 1) * 512],
                            lhsT=w1[p][:, kc, mc * 128:(mc + 1) * 128],
                            rhs=elu3[p][:, kc, nk * 512:(nk + 1) * 512],
                            start=(kc == 0), stop=(kc == 1))
                elu_chunk(e1[p][:, mc, :], h1, bias_col=b1[p][:, mc:mc + 1])

        # --- stage 2: h2 = W2 @ e1 + b2 ; gr = x + 0.5(a+b2a)(1+tanh(0.5(g+b2g))) ---
        gr = {p: big.tile([128, 2, S], mdt, name=f"gr_{p}") for p in BR}
        for p in BR:
            for cc in range(2):
                ps_a = psum_main.tile([128, 1024], f32, tag="pm")
                a_raw = ps_a[:, 0:S]
                for nk in range(2):
                    for kc in range(2):
                        nc.tensor.matmul(
                            a_raw[:, nk * 512:(nk + 1) * 512],
                            lhsT=w2[p][:, kc, cc * 128:(cc + 1) * 128],
                            rhs=e1[p][:, kc, nk * 512:(nk + 1) * 512],
                            start=(kc == 0), stop=(kc == 1))
                ps_g = psum_main.tile([128, 1024], f32, tag="pm")
                g_raw = ps_g[:, 0:S]
                for nk in range(2):
                    for kc in range(2):
                        nc.tensor.matmul(
                            g_raw[:, nk * 512:(nk + 1) * 512],
                            lhsT=w2[p][:, kc, (2 + cc) * 128:(3 + cc) * 128],
                            rhs=e1[p][:, kc, nk * 512:(nk + 1) * 512],
                            start=(kc == 0), stop=(kc == 1))
                ha = wk.tile([128, S], mdt, tag="wkr")
                tg = wk.tile([128, S], mdt, tag="wke")
                u = wk.tile([128, S], mdt, tag="wkm")
                nc.vector.tensor_scalar(ha, a_raw, b2[p][:, cc:cc + 1], 0.5,
                                        Op.add, Op.mult)
                nc.scalar.activation(tg, g_raw, AF.Tanh,
                                     bias=b2h[p][:, 2 + cc:3 + cc], scale=0.5)
                nc.vector.scalar_tensor_tensor(
                    u, tg, 1.0, ha, Op.add, Op.mult)
                eng(cfg["gr_add_engine"]).tensor_tensor(
                    gr[p][:, cc, :], u, x3[p][:, cc, :], Op.add)

        # --- stage 3: nin ---
        # v first (feeds v_aug via sbuf->sbuf DMA), then k/q interleaved by
        # hw-block pairs so tp chunks complete in order 0,1,2,3.
        v_sb = big.tile([128, 4, S], mdt, name="v_sb")
        for mc in range(4):
            ps = psum_main.tile([128, 1024], f32, tag="pm")
            vo = ps[:, 0:S]
            for nk in range(2):
                for kc in range(2):
                    nc.tensor.matmul(
                        vo[:, nk * 512:(nk + 1) * 512],
                        lhsT=wn["v"][:, kc, mc * 128:(mc + 1) * 128],
                        rhs=gr["v"][:, kc, nk * 512:(nk + 1) * 512],
                        start=(kc == 0), stop=(kc == 1))
            nc.vector.tensor_scalar(v_sb[:, mc, :], vo, bnv[:, mc:mc + 1], 0.0,
                                    Op.add, Op.add)
            # v_aug[j][p2, n, u] = V_att[128j+p2, 64n+u]; V_att[s, d] =
            # v_cm[s//2, (s%2)*512 + d].  j blocks 2*mc, 2*mc+1 live in chunk mc.
            for j in (2 * mc, 2 * mc + 1):
                src = v_sb[64 * (j % 2):64 * (j % 2) + 64, mc, :]
                src = src.rearrange("c (h n u) -> c h n u", h=2, n=NH)
                nc.sync.dma_start(out=v_aug[:, j, :, 0:VS], in_=src)

        def nin_T(p, hw_p):
            tgt = qT_m if p == "q" else kT_m
            ps = psum_main.tile([128, 1024], f32, tag="pm")
            oT = ps[:, 0:D]
            for kc in range(2):
                nc.tensor.matmul(
                    oT,
                    lhsT=gr[p][:, kc, hw_p * 128:(hw_p + 1) * 128],
                    rhs=wn[p][:, kc, :],
                    start=(kc == 0), stop=(kc == 1))
            tp, jj = hw_p % 4, hw_p // 4
            nc.vector.scalar_tensor_tensor(
                tgt[:, tp, jj::2], oT, 1.0, bnb[p], Op.mult, Op.add)

        for hw_p in (0, 4, 1, 5, 2, 6, 3, 7):
            nin_T("k", hw_p)
            nin_T("q", hw_p)

        # ---------------- attention ----------------
        # scores psum groups (each <= 1024 cols = 2 banks)
        GROUPS = [(0,), (1, 7), (2, 6), (3, 5), (4,)]
        G = {}
        off = 0
        for grp in GROUPS:
            for j in grp:
                G[j] = off
                off += S - 128 * j

        for n in range(NH):
            tp, po = n // 2, 64 * (n % 2)
            eT = eT_pool.tile([128, 4608], mdt, tag="eT")
            for grp in GROUPS:
                glen = sum(S - 128 * j for j in grp)
                gbase = G[grp[0]]
                ps = psum_main.tile([128, 1024], f32, tag="pm")
                for j in grp:
                    off = G[j] - gbase
                    lhsT = kT_m[po:po + 64, tp, 128 * j:128 * (j + 1)]
                    for s1a, s1b in _split_psum_ranges(off, off + (S - 128 * j)):
                        nc.tensor.matmul(
                            ps[:, s1a:s1b],
                            lhsT=lhsT,
                            rhs=qT_m[po:po + 64, tp,
                                     128 * j + (s1a - off):128 * j + (s1b - off)],
                            start=True, stop=True)
                nc.scalar.activation(eT[:, gbase:gbase + glen],
                                     ps[:, 0:glen], AF.Exp, scale=SCALE)
                for j in grp:
                    eng(cfg["mask_engine"]).tensor_tensor(
                        eT[:, G[j]:G[j] + 128], eT[:, G[j]:G[j] + 128],
                        mask01, Op.mult)

            for c in range(2):
                pv = psum_pv.tile([VS + 1, 512], f32, tag="pv")
                jmax = 3 if c == 0 else 7
                for j in range(jmax + 1):
                    s1a = max(512 * c, 128 * j)
                    s1b = 512 * (c + 1)
                    nc.tensor.matmul(
                        pv[:, s1a - 512 * c:512],
                        lhsT=v_aug[:, j, n, :],
                        rhs=eT[:, G[j] + (s1a - 128 * j):G[j] + (s1b - 128 * j)],
                        start=(j == 0), stop=(j == jmax))
                if c == 0:
                    nc.vector.memset(pv[VS:VS + 1, 0:1], 1.0)
                # normalize: 1/l -> broadcast to 64 partitions -> multiply
                rrow = att_small.tile([1, 512], f32, tag="rrow")
                nc.vector.reciprocal(rrow, pv[VS:VS + 1, :])
                rb = att_small.tile([VS, 512], f32, tag="rb")
                if cfg["bcast"] == "gpsimd":
                    nc.gpsimd.partition_broadcast(rb, rrow, channels=VS)
                else:
                    rsrc = bass.AP(tensor=rrow.tensor, offset=rrow.offset,
                                   ap=[[0, VS]] + list(rrow.ap)[1:])
                    nc.sync.dma_start(out=rb, in_=rsrc)
                fin = att_small.tile([VS, 512], f32, tag="fin")
                eng(cfg["fin_engine"]).tensor_tensor(
                    fin, pv[0:VS, :], rb, Op.mult)
                nc.sync.dma_start(
                    out=out_d[VS * n:VS * (n + 1), 512 * c:512 * (c + 1)],
                    in_=fin)

    nc.compile()
    return nc


_CACHE = {}


def _get_program(cfg_key=None):
    key = cfg_key or "default"
    if key not in _CACHE:
        _CACHE[key] = build_program(CFG)
    return _CACHE[key]


def make_in_map(inp, b):
    """Per-core input dict for batch b (weights host-transposed/cast)."""
    if CFG["mm_dtype"] == "bfloat16":
        import ml_dtypes
        wt = np.dtype(ml_dtypes.bfloat16)
    else:
        wt = np.float32
    m = {
        "query": np.ascontiguousarray(inp["query"][b].reshape(C, S)).astype(wt),
        "key": np.ascontiguousarray(inp["key"][b].reshape(C, S)).astype(wt),
    }
    for p in ("q", "k", "v"):
        m[f"{p}_w1T"] = np.ascontiguousarray(inp[f"{p}_gr_w1"].T).astype(wt)
        m[f"{p}_w2T"] = np.ascontiguousarray(inp[f"{p}_gr_w2"].T).astype(wt)
        m[f"{p}_wnT"] = np.ascontiguousarray(inp[f"{p}_nin_w"].T).astype(wt)
        m[f"{p}_b1"] = inp[f"{p}_gr_b1"]
        m[f"{p}_b2"] = inp[f"{p}_gr_b2"]
        m[f"{p}_bn"] = inp[f"{p}_nin_b"]
    return m


def kernel(**inputs):
    from concourse.bass_utils import run_bass_kernel_spmd

    nc = _get_program()
    inp = {k: np.asarray(v, dtype=np.float32) for k, v in inputs.items()}

    in_maps = [make_in_map(inp, b) for b in range(N_CORES)]

    trace = bool(int(os.environ.get("BASS_KERNEL_TRACE", "0")))
    res = run_bass_kernel_spmd(nc, in_maps, core_ids=list(range(N_CORES)),
                               trace=trace)
    LAST_RUN["exec_time_ns"] = getattr(res, "exec_time_ns", None)
    LAST_RUN["results"] = res
    out = np.stack([res.results[i]["out"].reshape(D, 32, 32)
                    for i in range(N_CORES)])
    return out.astype(np.float32)


LAST_RUN = {}


if __name__ == "__main__":
    nc = build_program()
    print("compiled OK")


# revision 12
# speedup vs baseline: 2.3391x; 2.3391x over previous
"""Trainium2 Bass kernel for nn_CausalAttention (gated-resnet q/k/v projections
+ causal attention). Data-parallel over batch: 8 batches -> 8 NeuronCores.

Per-core computation (batch b), all fp32 storage:
  x_q = query[b] (C=256, S=1024)   x_k = key[b] (256, 1024)
  branch(p, x): e  = elu(x)
                h1 = W1 @ e + b1 ; e1 = elu(h1)
                h2 = W2 @ e1 + b2 ; a, g = split(h2)
                gr = x + a * sigmoid(g)
                o  = Wn @ gr + bn          (512, 1024) channel-major
  q = branch(q, x_q); k = branch(k, x_k); v = branch(v, x_k)
  att view: X_att[s, d] = X_cm[s//2, (s%2)*512 + d]  (flat reinterpretation)
  per head n (d = 64n..64n+63):
    scoresT[s2, s1] = sum_d K_att[s2,d] Q_att[s1,d]   (s2 causal blocks)
    eT = exp(scoresT/sqrt(512)) with strict-lower mask (s2 < s1)
    outT[vs, s1] = sum_s2 V_att[s2, 64n+vs] * eT[s2, s1] ; l[s1] = sum_s2 eT
    final[64n+vs, s1] = outT[vs, s1] / l[s1]   (row 0 of l patched to 1)

v2: engine-rebalanced + software-pipelined:
  - branches issued stage-interleaved (h1 v,k,q; e1 v,k,q; ...) so PE
    matmuls of one branch overlap DVE/ACT work of another
  - elu combine / glu mult / gr add / masks moved to gpsimd (was idle)
  - v_aug built by direct SBUF->SBUF DMA (no DRAM roundtrip)
  - softmax normalize: reciprocal (DVE) -> gpsimd partition_broadcast ->
    DVE multiply (no DRAM roundtrips, no big psum->sbuf copy)
"""

import os
import sys
import numpy as np

sys.path.insert(0, "/opt/trn_rl_repo")

C = 256
S = 1024
D = 512
NH = 8
KS = 64
VS = 64
SCALE = 1.0 / float(np.sqrt(512.0))
N_CORES = 8

CFG = {
    "mm_dtype": "bfloat16",  # "float32" | "bfloat16"
    # gpsimd only supports plain tensor_tensor (no scalar-imm ops)
    "elu_combine_engine": "gpsimd",  # dst = me' + r   (me' = min(e,1)-1)
    "glu_mult_engine": "vector",     # u = ha*(1+tg)   (stt needs V)
    "gr_add_engine": "gpsimd",       # gr = u + x
    "mask_engine": "gpsimd",         # eT diag *= mask01
    "fin_engine": "vector",          # fin = pv * rb
    "bcast": "gpsimd",               # rb broadcast: "gpsimd" | "dma"
}


def _split_psum_ranges(a, b, max_n=512):
    """Split [a, b) psum column range into chunks that don't cross 512-col
    bank boundaries and are <= max_n wide."""
    out = []
    while a < b:
        nxt = min(b, ((a // 512) + 1) * 512, a + max_n)
        out.append((a, nxt))
        a = nxt
    return out


def build_program(cfg=CFG):
    from contextlib import ExitStack

    import concourse.bacc as bacc
    import concourse.bass as bass
    import concourse.tile as tile
    from concourse import mybir
    from concourse.alu_op_type import AluOpType as Op

    f32 = mybir.dt.float32
    mmdt = getattr(mybir.dt, cfg["mm_dtype"])
    mdt = mmdt
    AF = mybir.ActivationFunctionType

    nc = bacc.Bacc("TRN2", target_bir_lowering=False, debug=False,
                   num_devices=N_CORES)

    # ---------------- DRAM parameters ----------------
    idt = mybir.dt.bfloat16 if cfg["mm_dtype"] == "bfloat16" else f32
    query = nc.dram_tensor("query", [C, S], idt, kind="ExternalInput").ap()
    key = nc.dram_tensor("key", [C, S], idt, kind="ExternalInput").ap()
    wT = {}
    bias = {}
    wdt = mdt if mdt == mybir.dt.bfloat16 else f32
    for p in ("q", "k", "v"):
        wT[p, 1] = nc.dram_tensor(f"{p}_w1T", [C, C], wdt, kind="ExternalInput").ap()
        wT[p, 2] = nc.dram_tensor(f"{p}_w2T", [C, 2 * C], wdt, kind="ExternalInput").ap()
        wT[p, "n"] = nc.dram_tensor(f"{p}_wnT", [C, D], wdt, kind="ExternalInput").ap()
        bias[p, 1] = nc.dram_tensor(f"{p}_b1", [C], f32, kind="ExternalInput").ap()
        bias[p, 2] = nc.dram_tensor(f"{p}_b2", [2 * C], f32, kind="ExternalInput").ap()
        bias[p, "n"] = nc.dram_tensor(f"{p}_bn", [D], f32, kind="ExternalInput").ap()
    out_d = nc.dram_tensor("out", [D, S], f32, kind="ExternalOutput").ap()

    def eng(name):
        return getattr(nc, name)

    BR = ("v", "k", "q")  # issue order within stages

    with tile.TileContext(nc) as tc, ExitStack() as ctx:
        # ------------- pools -------------
        persist = ctx.enter_context(tc.tile_pool(name="persist", bufs=1))
        psum_main = ctx.enter_context(tc.tile_pool(name="psum_main", bufs=3, space="PSUM"))
        psum_pv = ctx.enter_context(tc.tile_pool(name="psum_pv", bufs=2, space="PSUM"))
        wk = ctx.enter_context(tc.tile_pool(name="wk", bufs=5))
        big = ctx.enter_context(tc.tile_pool(name="big", bufs=1))
        eT_pool = ctx.enter_context(tc.tile_pool(name="eT", bufs=3))
        att_small = ctx.enter_context(tc.tile_pool(name="att_small", bufs=4))

        # persistent tiles
        xq = persist.tile([128, 2, S], idt)
        xk = persist.tile([128, 2, S], idt)
        eluq = persist.tile([128, 2, S], mdt)
        eluk = persist.tile([128, 2, S], mdt)
        qT_m = persist.tile([128, 4, S], mdt)   # Q^T_att: [dd%128, dd//128, s]
        kT_m = persist.tile([128, 4, S], mdt)
        v_aug = persist.tile([128, 8, NH, VS + 1], mdt)  # [s%128, s//128, n, vs|1]

        # PE warm-up: back-to-back matmuls on scratch data during DMA phase
        warm = persist.tile([128, 512], mdt, name="warm")
        nc.vector.memset(warm, 0.5)
        wps = psum_main.tile([128, 1024], f32, tag="pm", name="wps")
        for _ in range(18):
            nc.tensor.matmul(wps[:, 0:512], lhsT=warm[:, 0:128],
                             rhs=warm, start=True, stop=True)

        # inputs
        for cc in range(2):
            nc.sync.dma_start(out=xk[:, cc, :], in_=key[cc * 128:(cc + 1) * 128, :])
            nc.sync.dma_start(out=xq[:, cc, :], in_=query[cc * 128:(cc + 1) * 128, :])
        bnb = {"q": persist.tile([128, D], f32, name="bnb_q"),
               "k": persist.tile([128, D], f32, name="bnb_k")}
        for p in ("q", "k"):
            bn_ap = bias[p, "n"]
            bn_bcast = bass.AP(tensor=bn_ap.tensor, offset=bn_ap.offset,
                               ap=[[0, 128]] + list(bn_ap.ap))
            nc.sync.dma_start(out=bnb[p], in_=bn_bcast)
        # v_aug ones column (independent of everything; do during DMA phase)
        for j in range(8):
            nc.gpsimd.memset(v_aug[:, j, :, VS:VS + 1], 1.0)

        # ------- weights for all branches upfront (persistent) -------
        w1 = {}
        w2 = {}
        wn = {}
        b1 = {}
        b2 = {}
        b2h = {}
        for p in BR:
            w1[p] = persist.tile([128, 2, C], mdt, name=f"w1_{p}")
            w2[p] = persist.tile([128, 2, 2 * C], mdt, name=f"w2_{p}")
            wn[p] = persist.tile([128, 2, D], mdt, name=f"wn_{p}")

            def wcast(ap):
                return ap if ap.dtype == mdt else ap.bitcast(mdt)
            for kc in range(2):
                nc.sync.dma_start(out=w1[p][:, kc, :], in_=wcast(wT[p, 1][kc * 128:(kc + 1) * 128, :]))
                nc.sync.dma_start(out=w2[p][:, kc, :], in_=wcast(wT[p, 2][kc * 128:(kc + 1) * 128, :]))
                nc.sync.dma_start(out=wn[p][:, kc, :], in_=wcast(wT[p, "n"][kc * 128:(kc + 1) * 128, :]))
            b1[p] = persist.tile([128, 2], f32, name=f"b1_{p}")
            b2[p] = persist.tile([128, 4], f32, name=f"b2_{p}")
            b2h[p] = persist.tile([128, 4], f32, name=f"b2h_{p}")
            nc.sync.dma_start(out=b1[p], in_=bias[p, 1].rearrange("(kc p) -> p kc", p=128))
            nc.sync.dma_start(out=b2[p], in_=bias[p, 2].rearrange("(kc p) -> p kc", p=128))
            nc.vector.tensor_scalar_mul(b2h[p], b2[p], 0.5)
        bnv = persist.tile([128, 4], f32, name="bnv")
        nc.sync.dma_start(out=bnv, in_=bias["v", "n"].rearrange("(kc p) -> p kc", p=128))

        # ---------------- branch compute, stage-interleaved ----------------
        # elu(x) for inputs: xk first (feeds v and k), then xq
        def elu_chunk(dst, src_ap, bias_col=None):
            """dst = elu(src) for a (128, S) chunk. src may be psum.
            vector: r = max(src [+b], 0); scalar: e = exp(src [+b]);
            gpsimd: me = min(e, 1); dst = (me - 1) + r."""
            r = wk.tile([128, S], mdt, tag="wkr")
            e = wk.tile([128, S], mdt, tag="wke")
            me = wk.tile([128, S], mdt, tag="wkm")
            if bias_col is None:
                nc.vector.tensor_scalar_max(r, src_ap, 0.0)
                nc.scalar.activation(e, src_ap, AF.Exp)
            else:
                nc.vector.tensor_scalar(r, src_ap, bias_col, 0.0, Op.add, Op.max)
                nc.scalar.activation(e, src_ap, AF.Exp, bias=bias_col)
            # me = min(e, 1) - 1   (one vector op); dst = me + r (gpsimd)
            nc.vector.tensor_scalar(me, e, 1.0, -1.0, Op.min, Op.add)
            eng(cfg["elu_combine_engine"]).tensor_tensor(dst, me, r, Op.add)

        for cc in range(2):
            elu_chunk(eluk[:, cc, :], xk[:, cc, :])
        for cc in range(2):
            elu_chunk(eluq[:, cc, :], xq[:, cc, :])

        elu3 = {"v": eluk, "k": eluk, "q": eluq}
        x3 = {"v": xk, "k": xk, "q": xq}

        # --- stage 1: h1 = W1 @ elu(x) + b1 ; e1 = elu(h1) ---
        e1 = {p: big.tile([128, 2, S], mdt, name=f"e1_{p}") for p in BR}
        for p in BR:
            for mc in range(2):
                ps = psum_main.tile([128, 1024], f32, tag="pm")
                h1 = ps[:, 0:S]
                for nk in range(2):
                    for kc in range(2):
                        nc.tensor.matmul(
                            h1[:, nk * 512:(nk + 1) * 512],
                            lhsT=w1[p][:, kc, mc * 128:(mc + 1) * 128],
                            rhs=elu3[p][:, kc, nk * 512:(nk + 1) * 512],
                            start=(kc == 0), stop=(kc == 1))
                elu_chunk(e1[p][:, mc, :], h1, bias_col=b1[p][:, mc:mc + 1])

        # --- stage 2: h2 = W2 @ e1 + b2 ; gr = x + 0.5(a+b2a)(1+tanh(0.5(g+b2g))) ---
        gr = {p: big.tile([128, 2, S], mdt, name=f"gr_{p}") for p in BR}
        for p in BR:
            for cc in range(2):
                ps_a = psum_main.tile([128, 1024], f32, tag="pm")
                a_raw = ps_a[:, 0:S]
                for nk in range(2):
                    for kc in range(2):
                        nc.tensor.matmul(
                            a_raw[:, nk * 512:(nk + 1) * 512],
                            lhsT=w2[p][:, kc, cc * 128:(cc + 1) * 128],
                            rhs=e1[p][:, kc, nk * 512:(nk + 1) * 512],
                            start=(kc == 0), stop=(kc == 1))
                ps_g = psum_main.tile([128, 1024], f32, tag="pm")
                g_raw = ps_g[:, 0:S]
                for nk in range(2):
                    for kc in range(2):
                        nc.tensor.matmul(
                            g_raw[:, nk * 512:(nk + 1) * 512],
                            lhsT=w2[p][:, kc, (2 + cc) * 128:(3 + cc) * 128],
                            rhs=e1[p][:, kc, nk * 512:(nk + 1) * 512],
                            start=(kc == 0), stop=(kc == 1))
                ha = wk.tile([128, S], mdt, tag="wkr")
                tg = wk.tile([128, S], mdt, tag="wke")
                u = wk.tile([128, S], mdt, tag="wkm")
                nc.vector.tensor_scalar(ha, a_raw, b2[p][:, cc:cc + 1], 0.5,
                                        Op.add, Op.mult)
                nc.scalar.activation(tg, g_raw, AF.Tanh,
                                     bias=b2h[p][:, 2 + cc:3 + cc], scale=0.5)
                nc.vector.scalar_tensor_tensor(
                    u, tg, 1.0, ha, Op.add, Op.mult)
                eng(cfg["gr_add_engine"]).tensor_tensor(
                    gr[p][:, cc, :], u, x3[p][:, cc, :], Op.add)

        # --- stage 3: nin ---
        # v first (feeds v_aug via sbuf->sbuf DMA), then k/q interleaved by
        # hw-block pairs so tp chunks complete in order 0,1,2,3.
        v_sb = big.tile([128, 4, S], mdt, name="v_sb")
        for mc in range(4):
            ps = psum_main.tile([128, 1024], f32, tag="pm")
            vo = ps[:, 0:S]
            for nk in range(2):
                for kc in range(2):
                    nc.tensor.matmul(
                        vo[:, nk * 512:(nk + 1) * 512],
                        lhsT=wn["v"][:, kc, mc * 128:(mc + 1) * 128],
                        rhs=gr["v"][:, kc, nk * 512:(nk + 1) * 512],
                        start=(kc == 0), stop=(kc == 1))
            nc.vector.tensor_scalar(v_sb[:, mc, :], vo, bnv[:, mc:mc + 1], 0.0,
                                    Op.add, Op.add)
            # v_aug[j][p2, n, u] = V_att[128j+p2, 64n+u]; V_att[s, d] =
            # v_cm[s//2, (s%2)*512 + d].  j blocks 2*mc, 2*mc+1 live in chunk mc.
            for j in (2 * mc, 2 * mc + 1):
                src = v_sb[64 * (j % 2):64 * (j % 2) + 64, mc, :]
                src = src.rearrange("c (h n u) -> c h n u", h=2, n=NH)
                nc.sync.dma_start(out=v_aug[:, j, :, 0:VS], in_=src)

        def nin_T(p, hw_p):
            tgt = qT_m if p == "q" else kT_m
            ps = psum_main.tile([128, 1024], f32, tag="pm")
            oT = ps[:, 0:D]
            for kc in range(2):
                nc.tensor.matmul(
                    oT,
                    lhsT=gr[p][:, kc, hw_p * 128:(hw_p + 1) * 128],
                    rhs=wn[p][:, kc, :],
                    start=(kc == 0), stop=(kc == 1))
            tp, jj = hw_p % 4, hw_p // 4
            nc.vector.scalar_tensor_tensor(
                tgt[:, tp, jj::2], oT, 1.0, bnb[p], Op.mult, Op.add)

        for hw_p in (0, 4, 1, 5, 2, 6, 3, 7):
            nin_T("k", hw_p)
            nin_T("q", hw_p)

        # ---------------- attention ----------------
        # scores psum groups (each <= 1024 cols = 2 banks)
        GROUPS = [(0,), (1, 7), (2, 6), (3, 5), (4,)]
        G = {}
        off = 0
        for grp in GROUPS:
            for j in grp:
                G[j] = off
                off += S - 128 * j

        for n in range(NH):
            tp, po = n // 2, 64 * (n % 2)
            eT = eT_pool.tile([128, 4608], mdt, tag="eT")
            for grp in GROUPS:
                glen = sum(S - 128 * j for j in grp)
                gbase = G[grp[0]]
                ps = psum_main.tile([128, 1024], f32, tag="pm")
                for j in grp:
                    off = G[j] - gbase
                    lhsT = kT_m[po:po + 64, tp, 128 * j:128 * (j + 1)]
                    for s1a, s1b in _split_psum_ranges(off, off + (S - 128 * j)):
                        nc.tensor.matmul(
                            ps[:, s1a:s1b],
                            lhsT=lhsT,
                            rhs=qT_m[po:po + 64, tp,
                                     128 * j + (s1a - off):128 * j + (s1b - off)],
                            start=True, stop=True)
                nc.scalar.activation(eT[:, gbase:gbase + glen],
                                     ps[:, 0:glen], AF.Exp, scale=SCALE)
                for j in grp:
                    # strict-lower mask on the diagonal block, in place:
                    # keep where t1 - t2 - 1 >= 0 else 0
                    nc.gpsimd.affine_select(
                        out=eT[:, G[j]:G[j] + 128], in_=eT[:, G[j]:G[j] + 128],
                        compare_op=Op.is_ge, fill=0.0,
                        base=-1, pattern=[[1, 128]], channel_multiplier=-1)

            for c in range(2):
                pv = psum_pv.tile([VS + 1, 512], f32, tag="pv")
                jmax = 3 if c == 0 else 7
                for j in range(jmax + 1):
                    s1a = max(512 * c, 128 * j)
                    s1b = 512 * (c + 1)
                    nc.tensor.matmul(
                        pv[:, s1a - 512 * c:512],
                        lhsT=v_aug[:, j, n, :],
                        rhs=eT[:, G[j] + (s1a - 128 * j):G[j] + (s1b - 128 * j)],
                        start=(j == 0), stop=(j == jmax))
                if c == 0:
                    nc.vector.memset(pv[VS:VS + 1, 0:1], 1.0)
                # normalize: 1/l -> broadcast to 64 partitions -> multiply
                lrow = att_small.tile([1, 512], f32, tag="lrow")
                nc.vector.tensor_copy(lrow, pv[VS:VS + 1, :])
                rrow = att_small.tile([1, 512], f32, tag="rrow")
                nc.vector.reciprocal_approx_fast(rrow, lrow)
                rb = att_small.tile([VS, 512], f32, tag="rb")
                if cfg["bcast"] == "gpsimd":
                    nc.gpsimd.partition_broadcast(rb, rrow, channels=VS)
                else:
                    rsrc = bass.AP(tensor=rrow.tensor, offset=rrow.offset,
                                   ap=[[0, VS]] + list(rrow.ap)[1:])
                    nc.sync.dma_start(out=rb, in_=rsrc)
                fin = att_small.tile([VS, 512], f32, tag="fin")
                eng(cfg["fin_engine"]).tensor_tensor(
                    fin, pv[0:VS, :], rb, Op.mult)
                nc.sync.dma_start(
                    out=out_d[VS * n:VS * (n + 1), 512 * c:512 * (c + 1)],
                    in_=fin)

    nc.compile()
    return nc


_CACHE = {}


def _get_program(cfg_key=None):
    key = cfg_key or "default"
    if key not in _CACHE:
        _CACHE[key] = build_program(CFG)
    return _CACHE[key]


def make_in_map(inp, b):
    """Per-core input dict for batch b (weights host-transposed/cast)."""
    if CFG["mm_dtype"] == "bfloat16":
        import ml_dtypes
        wt = np.dtype(ml_dtypes.bfloat16)
    else:
        wt = np.float32
    m = {
        "query": np.ascontiguousarray(inp["query"][b].reshape(C, S)).astype(wt),
        "key": np.ascontiguousarray(inp["key"][b].reshape(C, S)).astype(wt),
    }
    for p in ("q", "k", "v"):
        m[f"{p}_w1T"] = np.ascontiguousarray(inp[f"{p}_gr_w1"].T).astype(wt)
        m[f"{p}_w2T"] = np.ascontiguousarray(inp[f"{p}_gr_w2"].T).astype(wt)
        m[f"{p}_wnT"] = np.ascontiguousarray(inp[f"{p}_nin_w"].T).astype(wt)
        m[f"{p}_b1"] = inp[f"{p}_gr_b1"]
        m[f"{p}_b2"] = inp[f"{p}_gr_b2"]
        m[f"{p}_bn"] = inp[f"{p}_nin_b"]
    return m


def kernel(**inputs):
    from concourse.bass_utils import run_bass_kernel_spmd

    nc = _get_program()
    inp = {k: np.asarray(v, dtype=np.float32) for k, v in inputs.items()}

    in_maps = [make_in_map(inp, b) for b in range(N_CORES)]

    trace = bool(int(os.environ.get("BASS_KERNEL_TRACE", "0")))
    res = run_bass_kernel_spmd(nc, in_maps, core_ids=list(range(N_CORES)),
                               trace=trace)
    LAST_RUN["exec_time_ns"] = getattr(res, "exec_time_ns", None)
    LAST_RUN["results"] = res
    out = np.stack([res.results[i]["out"].reshape(D, 32, 32)
                    for i in range(N_CORES)])
    return out.astype(np.float32)


LAST_RUN = {}


if __name__ == "__main__":
    nc = build_program()
    print("compiled OK")
